# revision 26
# baseline (speedup 1.0000x reference)
"""Trainium2 Bass kernel for nn_MaskToken (scatter_memory).

Reference semantics (B=4, L=4096, D=1024, NUM_KEEP=1024):
  outputs_dropped[b, k, :] = inputs[b, idx_keep[k], :]          # gather
  outputs_masked[b, l, :]  = inputs[b, l, :] if l in idx_keep else mask_embedding
  mask_drop[l]             = 0.0 if l in idx_keep else 1.0
  idx_keep passthrough.

Strategy: shard the length axis across 8 cores (512 positions/core).
Each core's slice is laid out length-major ([512, B*D] = 512 groups of
16KB), so one DMA descriptor moves a length position for all 4 batches
at once — 4x fewer descriptors than row-granular movement, which is
what the Q7 descriptor-generation cost scales with. idx_keep is sorted,
so the kept positions of core c map to a contiguous span of
outputs_dropped — no collective needed. The device program is pure
indirect-DMA data movement (INDIRECT1D, mainline SWDGE ucode — no Q7
library load):

  1. 4 indirect gathers pull the kept groups (HBM -> SBUF), 128
     groups per instruction
  2. 4 indirect scatters write mask_embedding x4 into the dropped
     positions of outputs_masked (independent of the gathers; their
     descriptor prep and transfers overlap the gathers')
  3. per 128-token chunk, once its gather lands: one indirect scatter
     into packed outputs_dropped and one into the kept positions of
     outputs_masked

(2) and (3) write disjoint groups and together cover every group of
outputs_masked. Per-core counts are data-dependent; index lists are
padded with a sentinel that the DMA bounds check silently skips, so one
fixed SPMD program serves all cores.
"""

import numpy as np

import concourse.bacc as bacc
import concourse.bass as bass
import concourse.mybir as mybir
from concourse.bass_utils import run_bass_kernel_spmd

B, L, D = 4, 4096, 1024
NUM_KEEP = 1024
N_CORES = 8
LS = L // N_CORES            # 512 length groups per core
G = B * D                    # 4096 elements per group (16KB)
R = LS * B                   # 2048 rows of D per core
GC = LS // 128               # 4 token chunks of 128
SENT = np.int32(8192)        # OOB sentinel (> LS-1 -> bounds-check skip)

F32 = mybir.dt.float32
I32 = mybir.dt.int32

_BUILT = None


def _build():
    """One SPMD Bass program shared by all 8 cores."""
    nc = bacc.Bacc(None)
    # x is length-major: row l*B+b of [R, D] is inputs[b, lo+l, :]
    x = nc.dram_tensor("x", [R, D], F32, kind="ExternalInput")
    meb1 = nc.dram_tensor("meb1", [1, G], F32, kind="ExternalInput")
    # idx32[p, c]: token j=c*128+p of each list; SENT-padded.
    # cols [0:GC] = kept groups (gather src == om kept dst),
    # cols [GC:2*GC] = od dst (packed j), cols [2*GC:3*GC] = om dropped dst
    idx32 = nc.dram_tensor("idx32", [128, 3 * GC], I32, kind="ExternalInput")
    om = nc.dram_tensor("om", [R, D], F32, kind="ExternalOutput")
    od = nc.dram_tensor("od", [R, D], F32, kind="ExternalOutput")

    with (
        nc.Block() as block,
        nc.semaphore("ldi") as ldi,
        nc.semaphore("ldm") as ldm,
        nc.semaphore("g0") as g0,
        nc.semaphore("g1") as g1,
        nc.semaphore("g2") as g2,
        nc.semaphore("g3") as g3,
        nc.semaphore("sc") as sc,
        nc.semaphore("ms") as ms,
        nc.semaphore("mm") as mm,
        nc.semaphore("cp") as cp,
        nc.sbuf_tensor("xk", [128, GC, G], F32) as xk,
        nc.sbuf_tensor("mesb", [128, G], F32) as mesb,
        nc.sbuf_tensor("meb1sb", [1, G], F32) as meb1sb,
        nc.sbuf_tensor("ones", [1, 128], F32) as ones,
        nc.sbuf_tensor("i32sb", [128, 3 * GC], I32) as i32sb,
        nc.psum_tensor("psmm", [128, G], F32) as psmm,
    ):
        gsem = [g0, g1, g2, g3]
        x_g = x[:].rearrange("(l b) d -> l (b d)", b=B)     # [LS, G]
        om_g = om[:].rearrange("(l b) d -> l (b d)", b=B)   # [LS, G]
        od_g = od[:].rearrange("(g b) d -> g (b d)", b=B)   # [LS, G]

        @block.sync
        def _(sync):
            sync.dma_start(out=i32sb[:], in_=idx32[:]).then_inc(ldi, 16)
            sync.dma_start(out=meb1sb[:], in_=meb1[:]).then_inc(ldm, 16)

        @block.vector
        def _(v):
            v.memset(ones[:], 1.0).then_inc(ms, 1)
            for j in range(8):
                v.wait_ge(mm, j + 1)
                v.tensor_copy(
                    out=mesb[:, j * 512:(j + 1) * 512],
                    in_=psmm[:, j * 512:(j + 1) * 512],
                ).then_inc(cp, 1)

        @block.tensor
        def _(t):
            t.wait_ge(ldm, 16)
            t.wait_ge(ms, 1)
            # broadcast me x B to all 128 partitions: ones[1,128].T @ me[1,G]
            for j in range(8):
                t.matmul(
                    out=psmm[:, j * 512:(j + 1) * 512],
                    lhsT=ones[:],
                    rhs=meb1sb[:, j * 512:(j + 1) * 512],
                    start=True,
                    stop=True,
                ).then_inc(mm, 1)

        @block.gpsimd
        def _(g):
            g.wait_ge(ldi, 16)
            with g.register("bc") as bc:
                g.reg_mov(bc, LS - 1)
                for c in range(GC):
                    g.indirect_dma_start(
                        out=xk[:, c:c + 1, :].squeeze(1),
                        out_offset=None,
                        in_=x_g,
                        in_offset=bass.IndirectOffsetOnAxis(
                            ap=i32sb[:, c:c + 1], axis=0),
                        bounds_check=bc,
                        oob_is_err=False,
                    ).then_inc(gsem[c], 16)
                for c in range(GC):
                    g.wait_ge(gsem[c], 16)
                    g.indirect_dma_start(
                        out=od_g,
                        out_offset=bass.IndirectOffsetOnAxis(
                            ap=i32sb[:, GC + c:GC + c + 1], axis=0),
                        in_=xk[:, c:c + 1, :].squeeze(1),
                        in_offset=None,
                        bounds_check=bc,
                        oob_is_err=False,
                    ).then_inc(sc, 16)
                    g.indirect_dma_start(
                        out=om_g,
                        out_offset=bass.IndirectOffsetOnAxis(
                            ap=i32sb[:, c:c + 1], axis=0),
                        in_=xk[:, c:c + 1, :].squeeze(1),
                        in_offset=None,
                        bounds_check=bc,
                        oob_is_err=False,
                    ).then_inc(sc, 16)
                    if c == 1:
                        # mask_embedding scatters go mid-stream: the on-chip
                        # broadcast is ready by now, and emitting their 6MiB
                        # before the last kept chunks shortens the drain tail
                        g.wait_ge(cp, 8)
                        for m in range(GC):
                            g.indirect_dma_start(
                                out=om_g,
                                out_offset=bass.IndirectOffsetOnAxis(
                                    ap=i32sb[:, 2 * GC + m:2 * GC + m + 1],
                                    axis=0),
                                in_=mesb[:],
                                in_offset=None,
                                bounds_check=bc,
                                oob_is_err=False,
                            ).then_inc(sc, 16)
                g.wait_ge(sc, 16 * 3 * GC)

    nc.compile()
    return nc


def get_program():
    global _BUILT
    if _BUILT is None:
        _BUILT = _build()
    return _BUILT


def make_core_inputs(inputs, mask_embedding, idx_keep):
    """Host-side sharding: length-major slice per core, index lists."""
    idx_keep = np.asarray(idx_keep)
    keep_starts = np.searchsorted(idx_keep, np.arange(0, L + LS, LS))
    me = np.asarray(mask_embedding, dtype=np.float32)
    meb1 = np.tile(me, (1, B)).astype(np.float32)     # [1, G]

    in_maps = []
    counts = []
    for c in range(N_CORES):
        lo, hi = c * LS, (c + 1) * LS
        kl = (idx_keep[keep_starts[c]:keep_starts[c + 1]] - lo).astype(np.int64)
        n_c = len(kl)
        drop_mask = np.ones(LS, dtype=bool)
        drop_mask[kl] = False
        dl = np.nonzero(drop_mask)[0]

        idx32 = np.full((128, 3 * GC), SENT, dtype=np.int32)
        jj = np.arange(n_c)
        idx32[jj % 128, jj // 128] = kl                 # kept: src == om dst
        idx32[jj % 128, GC + jj // 128] = jj            # od dst (packed)
        ii = np.arange(LS - n_c)
        idx32[ii % 128, 2 * GC + ii // 128] = dl        # om dropped dst

        # length-major layout: x_c[l*B+b] = inputs[b, lo+l]
        x_c = np.ascontiguousarray(
            inputs[:, lo:hi, :].transpose(1, 0, 2), dtype=np.float32
        ).reshape(R, D)
        in_maps.append({"x": x_c, "meb1": meb1, "idx32": idx32})
        counts.append(n_c)
    return in_maps, counts, keep_starts


def kernel(inputs, mask_embedding, idx_keep):
    inputs = np.asarray(inputs)
    mask_embedding = np.asarray(mask_embedding)
    idx_keep = np.asarray(idx_keep).astype(np.int32)

    nc = get_program()
    in_maps, counts, keep_starts = make_core_inputs(
        inputs, mask_embedding, idx_keep)

    res = run_bass_kernel_spmd(nc, in_maps, list(range(N_CORES)))

    outputs_masked = np.empty((B, L, D), dtype=np.float32)
    outputs_dropped = np.empty((B, NUM_KEEP, D), dtype=np.float32)
    for c in range(N_CORES):
        lo, hi = c * LS, (c + 1) * LS
        outputs_masked[:, lo:hi, :] = (
            res.results[c]["om"].reshape(LS, B, D).transpose(1, 0, 2))
        n_c = counts[c]
        k0 = keep_starts[c]
        outputs_dropped[:, k0:k0 + n_c, :] = (
            res.results[c]["od"].reshape(LS, B, D)[:n_c].transpose(1, 0, 2))

    mask_drop = np.ones(L, dtype=np.float32)
    mask_drop[idx_keep] = 0.0

    return outputs_dropped, outputs_masked, mask_drop, idx_keep


# revision 27
# speedup vs baseline: 1.0359x; 1.0359x over previous
"""Trainium2 Bass kernel for nn_MaskToken (scatter_memory).

Reference semantics (B=4, L=4096, D=1024, NUM_KEEP=1024):
  outputs_dropped[b, k, :] = inputs[b, idx_keep[k], :]          # gather
  outputs_masked[b, l, :]  = inputs[b, l, :] if l in idx_keep else mask_embedding
  mask_drop[l]             = 0.0 if l in idx_keep else 1.0
  idx_keep passthrough.

Strategy: shard the length axis across 8 cores (512 positions/core).
Each core's slice is laid out length-major ([512, B*D] = 512 groups of
16KB), so one DMA descriptor moves a length position for all 4 batches
at once — 4x fewer descriptors than row-granular movement, which is
what the Q7 descriptor-generation cost scales with. idx_keep is sorted,
so the kept positions of core c map to a contiguous span of
outputs_dropped — no collective needed. The device program is pure
indirect-DMA data movement (INDIRECT1D, mainline SWDGE ucode — no Q7
library load):

  1. 4 indirect gathers pull the kept groups (HBM -> SBUF), 128
     groups per instruction
  2. 4 indirect scatters write mask_embedding x4 into the dropped
     positions of outputs_masked (independent of the gathers; their
     descriptor prep and transfers overlap the gathers')
  3. per 128-token chunk, once its gather lands: one indirect scatter
     into packed outputs_dropped and one into the kept positions of
     outputs_masked

(2) and (3) write disjoint groups and together cover every group of
outputs_masked. Per-core counts are data-dependent; index lists are
padded with a sentinel that the DMA bounds check silently skips, so one
fixed SPMD program serves all cores.
"""

import numpy as np

import concourse.bacc as bacc
import concourse.bass as bass
import concourse.mybir as mybir
from concourse.bass_utils import run_bass_kernel_spmd

B, L, D = 4, 4096, 1024
NUM_KEEP = 1024
N_CORES = 8
LS = L // N_CORES            # 512 length groups per core
G = B * D                    # 4096 elements per group (16KB)
R = LS * B                   # 2048 rows of D per core
GC = LS // 128               # 4 token chunks of 128
SENT = np.int32(8192)        # OOB sentinel (> LS-1 -> bounds-check skip)

F32 = mybir.dt.float32
I32 = mybir.dt.int32

_BUILT = None


def _build():
    """One SPMD Bass program shared by all 8 cores."""
    nc = bacc.Bacc(None)
    # x is length-major: row l*B+b of [R, D] is inputs[b, lo+l, :]
    x = nc.dram_tensor("x", [R, D], F32, kind="ExternalInput")
    meb1 = nc.dram_tensor("meb1", [1, G], F32, kind="ExternalInput")
    # idx32[p, c]: token j=c*128+p of each list; SENT-padded.
    # cols [0:GC] = kept groups (gather src == om kept dst),
    # cols [GC:2*GC] = od dst (packed j), cols [2*GC:3*GC] = om dropped dst
    idx32 = nc.dram_tensor("idx32", [128, 3 * GC], I32, kind="ExternalInput")
    om = nc.dram_tensor("om", [R, D], F32, kind="ExternalOutput")
    od = nc.dram_tensor("od", [R, D], F32, kind="ExternalOutput")

    with (
        nc.Block() as block,
        nc.semaphore("ldi") as ldi,
        nc.semaphore("ldm") as ldm,
        nc.semaphore("g0") as g0,
        nc.semaphore("g1") as g1,
        nc.semaphore("g2") as g2,
        nc.semaphore("g3") as g3,
        nc.semaphore("sc") as sc,
        nc.semaphore("ms") as ms,
        nc.semaphore("mm") as mm,
        nc.semaphore("cp") as cp,
        nc.sbuf_tensor("xk", [128, GC, G], F32) as xk,
        nc.sbuf_tensor("mesb", [128, G], F32) as mesb,
        nc.sbuf_tensor("meb1sb", [1, G], F32) as meb1sb,
        nc.sbuf_tensor("ones", [1, 128], F32) as ones,
        nc.sbuf_tensor("i32sb", [128, 3 * GC], I32) as i32sb,
        nc.psum_tensor("psmm", [128, G], F32) as psmm,
    ):
        gsem = [g0, g1, g2, g3]
        x_g = x[:].rearrange("(l b) d -> l (b d)", b=B)     # [LS, G]
        om_g = om[:].rearrange("(l b) d -> l (b d)", b=B)   # [LS, G]
        od_g = od[:].rearrange("(g b) d -> g (b d)", b=B)   # [LS, G]

        @block.sync
        def _(sync):
            sync.dma_start(out=i32sb[:], in_=idx32[:]).then_inc(ldi, 16)
            sync.dma_start(out=meb1sb[:], in_=meb1[:]).then_inc(ldm, 16)

        @block.vector
        def _(v):
            v.memset(ones[:], 1.0).then_inc(ms, 1)
            for j in range(8):
                v.wait_ge(mm, j + 1)
                v.tensor_copy(
                    out=mesb[:, j * 512:(j + 1) * 512],
                    in_=psmm[:, j * 512:(j + 1) * 512],
                ).then_inc(cp, 1)

        @block.tensor
        def _(t):
            t.wait_ge(ldm, 16)
            t.wait_ge(ms, 1)
            # broadcast me x B to all 128 partitions: ones[1,128].T @ me[1,G]
            for j in range(8):
                t.matmul(
                    out=psmm[:, j * 512:(j + 1) * 512],
                    lhsT=ones[:],
                    rhs=meb1sb[:, j * 512:(j + 1) * 512],
                    start=True,
                    stop=True,
                ).then_inc(mm, 1)

        @block.gpsimd
        def _(g):
            g.wait_ge(ldi, 16)
            with g.register("bc") as bc:
                g.reg_mov(bc, LS - 1)
                for c in range(GC):
                    g.indirect_dma_start(
                        out=xk[:, c:c + 1, :].squeeze(1),
                        out_offset=None,
                        in_=x_g,
                        in_offset=bass.IndirectOffsetOnAxis(
                            ap=i32sb[:, c:c + 1], axis=0),
                        bounds_check=bc,
                        oob_is_err=False,
                    ).then_inc(gsem[c], 16)
                for c in range(GC):
                    g.wait_ge(gsem[c], 16)
                    g.indirect_dma_start(
                        out=od_g,
                        out_offset=bass.IndirectOffsetOnAxis(
                            ap=i32sb[:, GC + c:GC + c + 1], axis=0),
                        in_=xk[:, c:c + 1, :].squeeze(1),
                        in_offset=None,
                        bounds_check=bc,
                        oob_is_err=False,
                    ).then_inc(sc, 16)
                    g.indirect_dma_start(
                        out=om_g,
                        out_offset=bass.IndirectOffsetOnAxis(
                            ap=i32sb[:, c:c + 1], axis=0),
                        in_=xk[:, c:c + 1, :].squeeze(1),
                        in_offset=None,
                        bounds_check=bc,
                        oob_is_err=False,
                    ).then_inc(sc, 16)
                g.wait_ge(cp, 8)
                for m in range(GC):
                    g.indirect_dma_start(
                        out=om_g,
                        out_offset=bass.IndirectOffsetOnAxis(
                            ap=i32sb[:, 2 * GC + m:2 * GC + m + 1], axis=0),
                        in_=mesb[:],
                        in_offset=None,
                        bounds_check=bc,
                        oob_is_err=False,
                    ).then_inc(sc, 16)
                g.wait_ge(sc, 16 * 3 * GC)

    nc.compile()
    return nc


def get_program():
    global _BUILT
    if _BUILT is None:
        _BUILT = _build()
    return _BUILT


def make_core_inputs(inputs, mask_embedding, idx_keep):
    """Host-side sharding: length-major slice per core, index lists."""
    idx_keep = np.asarray(idx_keep)
    keep_starts = np.searchsorted(idx_keep, np.arange(0, L + LS, LS))
    me = np.asarray(mask_embedding, dtype=np.float32)
    meb1 = np.tile(me, (1, B)).astype(np.float32)     # [1, G]

    in_maps = []
    counts = []
    for c in range(N_CORES):
        lo, hi = c * LS, (c + 1) * LS
        kl = (idx_keep[keep_starts[c]:keep_starts[c + 1]] - lo).astype(np.int64)
        n_c = len(kl)
        drop_mask = np.ones(LS, dtype=bool)
        drop_mask[kl] = False
        dl = np.nonzero(drop_mask)[0]

        idx32 = np.full((128, 3 * GC), SENT, dtype=np.int32)
        jj = np.arange(n_c)
        idx32[jj % 128, jj // 128] = kl                 # kept: src == om dst
        idx32[jj % 128, GC + jj // 128] = jj            # od dst (packed)
        ii = np.arange(LS - n_c)
        idx32[ii % 128, 2 * GC + ii // 128] = dl        # om dropped dst

        # length-major layout: x_c[l*B+b] = inputs[b, lo+l]
        x_c = np.ascontiguousarray(
            inputs[:, lo:hi, :].transpose(1, 0, 2), dtype=np.float32
        ).reshape(R, D)
        in_maps.append({"x": x_c, "meb1": meb1, "idx32": idx32})
        counts.append(n_c)
    return in_maps, counts, keep_starts


def kernel(inputs, mask_embedding, idx_keep):
    inputs = np.asarray(inputs)
    mask_embedding = np.asarray(mask_embedding)
    idx_keep = np.asarray(idx_keep).astype(np.int32)

    nc = get_program()
    in_maps, counts, keep_starts = make_core_inputs(
        inputs, mask_embedding, idx_keep)

    res = run_bass_kernel_spmd(nc, in_maps, list(range(N_CORES)))

    outputs_masked = np.empty((B, L, D), dtype=np.float32)
    outputs_dropped = np.empty((B, NUM_KEEP, D), dtype=np.float32)
    for c in range(N_CORES):
        lo, hi = c * LS, (c + 1) * LS
        outputs_masked[:, lo:hi, :] = (
            res.results[c]["om"].reshape(LS, B, D).transpose(1, 0, 2))
        n_c = counts[c]
        k0 = keep_starts[c]
        outputs_dropped[:, k0:k0 + n_c, :] = (
            res.results[c]["od"].reshape(LS, B, D)[:n_c].transpose(1, 0, 2))

    mask_drop = np.ones(L, dtype=np.float32)
    mask_drop[idx_keep] = 0.0

    return outputs_dropped, outputs_masked, mask_drop, idx_keep


# revision 29
# speedup vs baseline: 1.0490x; 1.0127x over previous
"""Trainium2 Bass kernel for nn_MaskToken (scatter_memory).

Reference semantics (B=4, L=4096, D=1024, NUM_KEEP=1024):
  outputs_dropped[b, k, :] = inputs[b, idx_keep[k], :]          # gather
  outputs_masked[b, l, :]  = inputs[b, l, :] if l in idx_keep else mask_embedding
  mask_drop[l]             = 0.0 if l in idx_keep else 1.0
  idx_keep passthrough.

Strategy: shard the length axis across 8 cores (512 positions/core).
Each core's slice is laid out length-major ([512, B*D] = 512 groups of
16KB), so one DMA descriptor moves a length position for all 4 batches
at once — 4x fewer descriptors than row-granular movement, which is
what the Q7 descriptor-generation cost scales with. idx_keep is sorted,
so the kept positions of core c map to a contiguous span of
outputs_dropped — no collective needed. The device program is pure
indirect-DMA data movement (INDIRECT1D, mainline SWDGE ucode — no Q7
library load):

  1. 4 indirect gathers pull the kept groups (HBM -> SBUF), 128
     groups per instruction
  2. 4 indirect scatters write mask_embedding x4 into the dropped
     positions of outputs_masked (independent of the gathers; their
     descriptor prep and transfers overlap the gathers')
  3. per 128-token chunk, once its gather lands: one indirect scatter
     into packed outputs_dropped and one into the kept positions of
     outputs_masked

(2) and (3) write disjoint groups and together cover every group of
outputs_masked. Per-core counts are data-dependent; index lists are
padded with a sentinel that the DMA bounds check silently skips, so one
fixed SPMD program serves all cores.
"""

import numpy as np

import concourse.bacc as bacc
import concourse.bass as bass
import concourse.mybir as mybir
from concourse.bass_utils import run_bass_kernel_spmd

B, L, D = 4, 4096, 1024
NUM_KEEP = 1024
N_CORES = 8
LS = L // N_CORES            # 512 length groups per core
G = B * D                    # 4096 elements per group (16KB)
R = LS * B                   # 2048 rows of D per core
GC = LS // 128               # 4 token chunks of 128
SENT = np.int32(8192)        # OOB sentinel (> LS-1 -> bounds-check skip)

F32 = mybir.dt.float32
I32 = mybir.dt.int32

_BUILT = None


def _build():
    """One SPMD Bass program shared by all 8 cores."""
    nc = bacc.Bacc(None)
    # x is length-major: row l*B+b of [R, D] is inputs[b, lo+l, :]
    x = nc.dram_tensor("x", [R, D], F32, kind="ExternalInput")
    meb1 = nc.dram_tensor("meb1", [1, D], F32, kind="ExternalInput")
    # idx32[p, c]: token j=c*128+p of each list; SENT-padded.
    # cols [0:GC] = kept groups (gather src == om kept dst),
    # cols [GC:2*GC] = od dst (packed j), cols [2*GC:3*GC] = om dropped dst
    idx32 = nc.dram_tensor("idx32", [128, 3 * GC], I32, kind="ExternalInput")
    om = nc.dram_tensor("om", [R, D], F32, kind="ExternalOutput")
    od = nc.dram_tensor("od", [R, D], F32, kind="ExternalOutput")

    with (
        nc.Block() as block,
        nc.semaphore("ldi") as ldi,
        nc.semaphore("ldm") as ldm,
        nc.semaphore("g0") as g0,
        nc.semaphore("g1") as g1,
        nc.semaphore("g2") as g2,
        nc.semaphore("g3") as g3,
        nc.semaphore("sc") as sc,
        nc.semaphore("ms") as ms,
        nc.semaphore("mm") as mm,
        nc.semaphore("cp") as cp,
        nc.sbuf_tensor("xk", [128, GC, G], F32) as xk,
        nc.sbuf_tensor("mesb", [128, G], F32) as mesb,
        nc.sbuf_tensor("meb1sb", [1, D], F32) as meb1sb,
        nc.sbuf_tensor("ones", [1, 128], F32) as ones,
        nc.sbuf_tensor("i32sb", [128, 3 * GC], I32) as i32sb,
        nc.psum_tensor("psmm", [128, D], F32) as psmm,
    ):
        gsem = [g0, g1, g2, g3]
        x_g = x[:].rearrange("(l b) d -> l (b d)", b=B)     # [LS, G]
        om_g = om[:].rearrange("(l b) d -> l (b d)", b=B)   # [LS, G]
        od_g = od[:].rearrange("(g b) d -> g (b d)", b=B)   # [LS, G]

        @block.sync
        def _(sync):
            sync.dma_start(out=i32sb[:], in_=idx32[:]).then_inc(ldi, 16)
            sync.dma_start(out=meb1sb[:], in_=meb1[:]).then_inc(ldm, 16)

        @block.vector
        def _(v):
            v.memset(ones[:], 1.0).then_inc(ms, 1)
            # replicate the broadcast D-vector into all four 4KB quarters of
            # the 16KB group: bank-0 consumers first, then bank-1
            for k in (0, 2, 4, 6, 1, 3, 5, 7):
                v.wait_ge(mm, (k % 2) + 1)
                v.tensor_copy(
                    out=mesb[:, k * 512:(k + 1) * 512],
                    in_=psmm[:, (k % 2) * 512:((k % 2) + 1) * 512],
                ).then_inc(cp, 1)

        @block.tensor
        def _(t):
            t.wait_ge(ldm, 16)
            t.wait_ge(ms, 1)
            # broadcast me (D=1024, 2 PSUM banks) to all 128 partitions:
            # ones[1,128].T @ me[1,512]
            for j in range(2):
                t.matmul(
                    out=psmm[:, j * 512:(j + 1) * 512],
                    lhsT=ones[:],
                    rhs=meb1sb[:, j * 512:(j + 1) * 512],
                    start=True,
                    stop=True,
                ).then_inc(mm, 1)

        @block.gpsimd
        def _(g):
            g.wait_ge(ldi, 16)
            with g.register("bc") as bc:
                g.reg_mov(bc, LS - 1)
                for c in range(GC):
                    g.indirect_dma_start(
                        out=xk[:, c:c + 1, :].squeeze(1),
                        out_offset=None,
                        in_=x_g,
                        in_offset=bass.IndirectOffsetOnAxis(
                            ap=i32sb[:, c:c + 1], axis=0),
                        bounds_check=bc,
                        oob_is_err=False,
                    ).then_inc(gsem[c], 16)
                for c in range(GC):
                    g.wait_ge(gsem[c], 16)
                    g.indirect_dma_start(
                        out=od_g,
                        out_offset=bass.IndirectOffsetOnAxis(
                            ap=i32sb[:, GC + c:GC + c + 1], axis=0),
                        in_=xk[:, c:c + 1, :].squeeze(1),
                        in_offset=None,
                        bounds_check=bc,
                        oob_is_err=False,
                    ).then_inc(sc, 16)
                    g.indirect_dma_start(
                        out=om_g,
                        out_offset=bass.IndirectOffsetOnAxis(
                            ap=i32sb[:, c:c + 1], axis=0),
                        in_=xk[:, c:c + 1, :].squeeze(1),
                        in_offset=None,
                        bounds_check=bc,
                        oob_is_err=False,
                    ).then_inc(sc, 16)
                g.wait_ge(cp, 8)
                for m in range(GC):
                    g.indirect_dma_start(
                        out=om_g,
                        out_offset=bass.IndirectOffsetOnAxis(
                            ap=i32sb[:, 2 * GC + m:2 * GC + m + 1], axis=0),
                        in_=mesb[:],
                        in_offset=None,
                        bounds_check=bc,
                        oob_is_err=False,
                    ).then_inc(sc, 16)
                g.wait_ge(sc, 16 * 3 * GC)

    nc.compile()
    return nc


def get_program():
    global _BUILT
    if _BUILT is None:
        _BUILT = _build()
    return _BUILT


def make_core_inputs(inputs, mask_embedding, idx_keep):
    """Host-side sharding: length-major slice per core, index lists."""
    idx_keep = np.asarray(idx_keep)
    keep_starts = np.searchsorted(idx_keep, np.arange(0, L + LS, LS))
    me = np.asarray(mask_embedding, dtype=np.float32)
    meb1 = me.reshape(1, D).astype(np.float32)

    in_maps = []
    counts = []
    for c in range(N_CORES):
        lo, hi = c * LS, (c + 1) * LS
        kl = (idx_keep[keep_starts[c]:keep_starts[c + 1]] - lo).astype(np.int64)
        n_c = len(kl)
        drop_mask = np.ones(LS, dtype=bool)
        drop_mask[kl] = False
        dl = np.nonzero(drop_mask)[0]

        idx32 = np.full((128, 3 * GC), SENT, dtype=np.int32)
        jj = np.arange(n_c)
        idx32[jj % 128, jj // 128] = kl                 # kept: src == om dst
        idx32[jj % 128, GC + jj // 128] = jj            # od dst (packed)
        ii = np.arange(LS - n_c)
        idx32[ii % 128, 2 * GC + ii // 128] = dl        # om dropped dst

        # length-major layout: x_c[l*B+b] = inputs[b, lo+l]
        x_c = np.ascontiguousarray(
            inputs[:, lo:hi, :].transpose(1, 0, 2), dtype=np.float32
        ).reshape(R, D)
        in_maps.append({"x": x_c, "meb1": meb1, "idx32": idx32})
        counts.append(n_c)
    return in_maps, counts, keep_starts


def kernel(inputs, mask_embedding, idx_keep):
    inputs = np.asarray(inputs)
    mask_embedding = np.asarray(mask_embedding)
    idx_keep = np.asarray(idx_keep).astype(np.int32)

    nc = get_program()
    in_maps, counts, keep_starts = make_core_inputs(
        inputs, mask_embedding, idx_keep)

    res = run_bass_kernel_spmd(nc, in_maps, list(range(N_CORES)))

    outputs_masked = np.empty((B, L, D), dtype=np.float32)
    outputs_dropped = np.empty((B, NUM_KEEP, D), dtype=np.float32)
    for c in range(N_CORES):
        lo, hi = c * LS, (c + 1) * LS
        outputs_masked[:, lo:hi, :] = (
            res.results[c]["om"].reshape(LS, B, D).transpose(1, 0, 2))
        n_c = counts[c]
        k0 = keep_starts[c]
        outputs_dropped[:, k0:k0 + n_c, :] = (
            res.results[c]["od"].reshape(LS, B, D)[:n_c].transpose(1, 0, 2))

    mask_drop = np.ones(L, dtype=np.float32)
    mask_drop[idx_keep] = 0.0

    return outputs_dropped, outputs_masked, mask_drop, idx_keep
